# revision 1
# baseline (speedup 1.0000x reference)
"""MoE expert-FFN kernel for Trainium2, expert-parallel across 8 NeuronCores.

Problem: out[t] = silu(x[t] @ W1[e_t]^T) @ W2[e_t]^T with
  E=64 experts, D=512, H=1024, T=256 tokens.

Strategy (memory-bound on expert weights, ~268MB fp32 total):
  - Core c owns experts [8c, 8c+8). Host routes tokens to the core owning
    their expert (the hint's all-to-all done on host since we hold full
    inputs), padding each expert's tokens to a fixed capacity C.
  - Host pre-packs weights into the exact SBUF layout so the device does
    nothing but stream 4MiB/expert with perfect 128-partition DMAs.
  - On device, weights are the MOVING matmul operand (N=512 columns,
    full-rate float32r) and the tiny token blocks are the stationary
    operand, so the PE streams each weight element exactly once:
       H = silu(W1T-tiles streamed against x^T)     [tok, 1024] in PSUM
       H^T via 8 PE-transposes                      [128, tok] chunks
       Y = W2T-tiles streamed against H^T           [tok, 512]
  - float32r: full 4-byte weights in HBM (memory regime unchanged) with
    single-pass PE streaming; ~1.8e-4 absmax-relative vs the fp32 oracle.
"""

import numpy as np

E, D, H, T = 64, 512, 1024, 256
NCORES = 8
EPC = E // NCORES          # experts per core
DC = D // 128              # 4 d-chunks
HC = H // 128              # 8 h-chunks
WCOLS = DC * H + HC * D    # 8192 free columns of packed weights per expert
CB = 32                    # token block (PE-transpose granularity)

_prog_cache = {}


def _build_program(C, w_bufs=6, wdt_name="f32r"):
    import concourse.mybir as mybir
    import concourse.tile as tile
    from concourse import bacc

    f32 = mybir.dt.float32
    wdt = {"f32": f32, "f32r": mybir.dt.float32r,
           "bf16": mybir.dt.bfloat16, "f16": mybir.dt.float16}[wdt_name]
    blocks = C // CB
    nc = bacc.Bacc("TRN2", target_bir_lowering=False, debug=False)

    wts = nc.dram_tensor("wts", [EPC, 128, WCOLS], wdt, kind="ExternalInput")
    xt = nc.dram_tensor("xt", [128, EPC * DC * C], wdt, kind="ExternalInput")
    idt = nc.dram_tensor("idt", [CB, CB], wdt, kind="ExternalInput")
    yt = nc.dram_tensor("yt", [EPC, blocks, CB, D], f32, kind="ExternalOutput")

    with tile.TileContext(nc) as tc:
        with (
            tc.tile_pool(name="wpool", bufs=w_bufs) as wpool,
            tc.tile_pool(name="xpool", bufs=1) as xpool,
            tc.tile_pool(name="cpool", bufs=1) as cpool,
            tc.tile_pool(name="hpool", bufs=2) as hpool,
            tc.tile_pool(name="ypool", bufs=2) as ypool,
            tc.tile_pool(name="psh", bufs=2, space="PSUM") as pshp,
            tc.tile_pool(name="pst", bufs=2, space="PSUM") as pstp,
            tc.tile_pool(name="psy", bufs=2, space="PSUM") as psyp,
        ):
            ident = cpool.tile([CB, CB], wdt)
            nc.sync.dma_start(ident[:], idt[:])
            ident_w = ident[:]
            xall = xpool.tile([128, EPC * DC * C], wdt)
            nc.sync.dma_start(xall[:], xt[:])

            for s in range(EPC):
                w1 = wpool.tile([128, DC * H], wdt, tag="w")
                nc.sync.dma_start(w1[:], wts[s][:, :DC * H])
                w2 = wpool.tile([128, HC * D], wdt, tag="w")
                nc.sync.dma_start(w2[:], wts[s][:, DC * H:])

                for b in range(blocks):
                    # ---- fc1: Hpre[t, h] = sum_d x^T[d, t] * W1T[d, h]
                    psh = pshp.tile([CB, H], f32, tag="psh")
                    for nh in range(2):
                        for c in range(DC):
                            nc.tensor.matmul(
                                psh[:, nh * 512:(nh + 1) * 512],
                                xall[:, (s * DC + c) * C + b * CB:
                                     (s * DC + c) * C + (b + 1) * CB],
                                w1[:, c * H + nh * 512: c * H + (nh + 1) * 512],
                                start=(c == 0),
                                stop=(c == DC - 1),
                            )

                    # ---- silu: h = psh * sigmoid(psh)   [CB, 1024] -> SBUF
                    sig = hpool.tile([CB, H], f32, tag="sig")
                    nc.scalar.activation(
                        sig[:], psh[:], mybir.ActivationFunctionType.Sigmoid
                    )
                    hbuf = hpool.tile([CB, H], wdt, tag="h")
                    nc.vector.tensor_mul(hbuf[:], psh[:], sig[:])

                    # ---- transpose h -> hT [128, HC*CB] via PE
                    pst = pstp.tile([128, HC * CB], wdt, tag="pst")
                    for ch in range(HC):
                        nc.tensor.transpose(
                            pst[:, ch * CB:(ch + 1) * CB],
                            hbuf[:, ch * 128:(ch + 1) * 128],
                            ident_w,
                        )
                    ht = hpool.tile([128, HC * CB], wdt, tag="ht")
                    nc.vector.tensor_copy(ht[:], pst[:])

                    # ---- fc2: Y[t, d] = sum_h hT[h, t] * W2T[h, d]
                    psy = psyp.tile([CB, D], f32, tag="psy")
                    for ch in range(HC):
                        nc.tensor.matmul(
                            psy[:],
                            ht[:, ch * CB:(ch + 1) * CB],
                            w2[:, ch * D: (ch + 1) * D],
                            start=(ch == 0),
                            stop=(ch == HC - 1),
                        )

                    ybuf = ypool.tile([CB, D], f32, tag="y")
                    nc.vector.tensor_copy(ybuf[:], psy[:])
                    nc.scalar.dma_start(yt[s, b], ybuf[:])

    nc.compile()
    return nc


def _route(expert_idx):
    idx = np.asarray(expert_idx).astype(np.int64)
    order = np.argsort(idx, kind="stable")
    counts = np.bincount(idx, minlength=E)
    starts = np.zeros(E + 1, dtype=np.int64)
    starts[1:] = np.cumsum(counts)
    return order, starts, counts


def _pack_inputs(x, fc1_w, fc2_w, order, starts, C, np_dtype=np.float32):
    in_maps = []
    for core in range(NCORES):
        wh = np.empty((EPC, 128, WCOLS), np_dtype)
        xh = np.zeros((128, EPC * DC * C), np_dtype)
        for s in range(EPC):
            e = core * EPC + s
            # W1^T = fc1_w[e].T : [D, H]; d = c*128 + p -> col c*H + h
            w1t = np.ascontiguousarray(fc1_w[e].T).reshape(DC, 128, H)
            wh[s, :, :DC * H] = w1t.transpose(1, 0, 2).reshape(128, DC * H)
            # W2^T = fc2_w[e].T : [H, D]; h = ch*128 + p -> col DC*H + ch*D + d
            w2t = np.ascontiguousarray(fc2_w[e].T).reshape(HC, 128, D)
            wh[s, :, DC * H:] = w2t.transpose(1, 0, 2).reshape(128, HC * D)

            toks = order[starts[e]:starts[e + 1]]
            n = len(toks)
            if n:
                xte = np.ascontiguousarray(x[toks].T).reshape(DC, 128, n)
                for c in range(DC):
                    base = (s * DC + c) * C
                    xh[:, base:base + n] = xte[c]
        in_maps.append({"wts": wh, "xt": xh,
                        "idt": np.eye(CB, dtype=np_dtype)})
    return in_maps


def _unpack_outputs(results, order, starts, C, out_dtype):
    out = np.zeros((T, D), out_dtype)
    for core in range(NCORES):
        yh = np.asarray(results[core]["yt"]).reshape(EPC, C, D)
        for s in range(EPC):
            e = core * EPC + s
            toks = order[starts[e]:starts[e + 1]]
            n = len(toks)
            if n:
                out[toks] = yh[s, :n]
    return out


def kernel(x, expert_idx, fc1_w, fc2_w):
    from concourse.bass_utils import run_bass_kernel_spmd

    x = np.asarray(x, dtype=np.float32)
    fc1_w = np.asarray(fc1_w, dtype=np.float32)
    fc2_w = np.asarray(fc2_w, dtype=np.float32)

    order, starts, counts = _route(expert_idx)
    C = max(CB, int(-(-int(counts.max()) // CB) * CB))

    if C not in _prog_cache:
        _prog_cache[C] = _build_program(C)
    nc = _prog_cache[C]

    in_maps = _pack_inputs(x, fc1_w, fc2_w, order, starts, C)
    res = run_bass_kernel_spmd(nc, in_maps, list(range(NCORES)))
    return _unpack_outputs(res.results, order, starts, C, np.float32)



# revision 16
# speedup vs baseline: 1.3818x; 1.3818x over previous
"""MoE expert-FFN kernel for Trainium2, expert-parallel across 8 NeuronCores.

Problem: out[t] = silu(x[t] @ W1[e_t]^T) @ W2[e_t]^T with
  E=64 experts, D=512, H=1024, T=256 tokens.

Memory-bound on expert weights. Strategy:
  - Core c owns experts [8c, 8c+8); host routes tokens (all-to-all on host),
    padding each expert's tokens to capacity C (=32 here).
  - Weights are stored in HBM as int8 (per-channel symmetric quantization),
    byte-paired into uint16 words: 8.4MB/core instead of 33.5MB fp32.
    Quantization scales never touch the device: W1's per-d scales are folded
    into the packed x copies, W2's per-d scales into host output unpacking.
  - On device the int8 pairs are unpacked to bf16 with two tensor_scalar ops
    per weight half (lsr/and + bias), eligible for DVE 2x/4x perf modes.
  - Matmuls run in bf16 with weights as the moving operand (N=512 streams).
    Four 32-token expert blocks are stacked on PSUM partitions (offsets
    0/32/64/96), which engages 128x32 column tiling so the four blocks'
    streams can overlap in the PE array. One ACT Silu per 4-expert quad,
    8 batched [128,128] PE transposes, then fc2 back to [tok, D].
"""

import numpy as np

E, D, H, T = 64, 512, 1024, 256
NCORES = 8
EPC = E // NCORES          # experts per core
DC = D // 128              # 4 d-chunks
HC = H // 128              # 8 h-chunks
CB = 32                    # token block (PSUM partition-stacking granularity)
QUAD = 4                   # token blocks per PSUM quad

# per-expert int8 weight columns: W1 4096 + W2 4096
WQCOLS = 8192
# engine split points for the int8->bf16 cast (DVE / ACT / Pool)
CAST_SPLITS = (0, 2816, 6144, 8192)

_prog_cache = {}


def _build_program(C):
    import concourse.mybir as mybir
    import concourse.tile as tile
    from concourse import bacc

    f32 = mybir.dt.float32
    bf16 = mybir.dt.bfloat16
    i8 = mybir.dt.int8
    Act = mybir.ActivationFunctionType

    blocks = C // CB
    nblk = EPC * blocks            # token blocks per core
    nquad = (nblk + QUAD - 1) // QUAD
    assert nblk % QUAD == 0, "token blocks must tile into quads"

    nc = bacc.Bacc("TRN2", target_bir_lowering=False, debug=False)

    wq = nc.dram_tensor("wq", [EPC, 128, WQCOLS], i8, kind="ExternalInput")
    xt = nc.dram_tensor("xt", [128, nblk * DC * CB], bf16, kind="ExternalInput")
    idt = nc.dram_tensor("idt", [128, 128], bf16, kind="ExternalInput")
    yt = nc.dram_tensor("yt", [nquad, 128, D], f32, kind="ExternalOutput")

    with tile.TileContext(nc) as tc:
        with (
            tc.tile_pool(name="wqpool", bufs=4) as wqpool,
            tc.tile_pool(name="wbpool", bufs=8) as wbpool,
            tc.tile_pool(name="xpool", bufs=1) as xpool,
            tc.tile_pool(name="cpool", bufs=1) as cpool,
            tc.tile_pool(name="hpool", bufs=2) as hpool,
            tc.tile_pool(name="ypool", bufs=2) as ypool,
            tc.tile_pool(name="psh", bufs=2, space="PSUM") as pshp,
            tc.tile_pool(name="pst", bufs=2, space="PSUM") as pstp,
            tc.tile_pool(name="psy", bufs=2, space="PSUM") as psyp,
        ):
            ident = cpool.tile([128, 128], bf16)
            nc.sync.dma_start(ident[:], idt[:])
            xall = xpool.tile([128, nblk * DC * CB], bf16)
            nc.sync.dma_start(xall[:], xt[:])

            wb_of = {}           # expert slot -> bf16 weight tile

            def load_expert(s):
                wqs = wqpool.tile([128, WQCOLS], i8, tag="wq")
                nc.sync.dma_start(wqs[:], wq[s])
                wb = wbpool.tile([128, WQCOLS], bf16, tag="wb")
                # int8 -> bf16 cast, columns split across three engines
                a, b, c, d = CAST_SPLITS
                nc.vector.tensor_copy(wb[:, a:b], wqs[:, a:b])
                nc.scalar.activation(wb[:, b:c], wqs[:, b:c], Act.Copy)
                nc.gpsimd.tensor_copy(wb[:, c:d], wqs[:, c:d])
                wb_of[s] = wb

            # token block tb = (expert slot s, block b); x block index
            tbs = [(s, b) for s in range(EPC) for b in range(blocks)]

            for q in range(nquad):
                qtbs = tbs[q * QUAD:(q + 1) * QUAD]
                for s, b in qtbs:
                    if s not in wb_of:
                        load_expert(s)

                # fc1: psh[32j+i, h] = sum_d xs[d, tok i of tb j] * W1T[d, h]
                psh = pshp.tile([128, H], f32, tag="psh")
                for dc in range(DC):
                    for nh in range(2):
                        for j, (s, b) in enumerate(qtbs):
                            xoff = ((s * blocks + b) * DC + dc) * CB
                            nc.tensor.matmul(
                                psh[32 * j:32 * (j + 1),
                                    nh * 512:(nh + 1) * 512],
                                xall[:, xoff:xoff + CB],
                                wb_of[s][:, dc * H + nh * 512:
                                         dc * H + (nh + 1) * 512],
                                start=(dc == 0),
                                stop=(dc == DC - 1),
                                tile_position=(0, 32 * j),
                            )

                # silu on the whole quad -> bf16
                hq = hpool.tile([128, H], bf16, tag="hq")
                nc.scalar.activation(hq[:], psh[:], Act.Silu)

                # transpose: hq [tokq, h] -> htq [h, tokq], 8 chunks of 128
                pst = pstp.tile([128, H], bf16, tag="pst")
                for hc in range(HC):
                    nc.tensor.transpose(
                        pst[:, hc * 128:(hc + 1) * 128],
                        hq[:, hc * 128:(hc + 1) * 128],
                        ident[:],
                    )
                htq = hpool.tile([128, H], bf16, tag="htq")
                nc.vector.tensor_copy(htq[:], pst[:])

                # fc2: psy[32j+i, d] = sum_h htq[h, 32j+i] * W2T[h, d]
                psy = psyp.tile([128, D], f32, tag="psy")
                for hc in range(HC):
                    for j, (s, b) in enumerate(qtbs):
                        nc.tensor.matmul(
                            psy[32 * j:32 * (j + 1), :],
                            htq[:, hc * 128 + 32 * j:hc * 128 + 32 * (j + 1)],
                            wb_of[s][:, 4096 + hc * D:4096 + (hc + 1) * D],
                            start=(hc == 0),
                            stop=(hc == HC - 1),
                            tile_position=(0, 32 * j),
                        )

                yq = ypool.tile([128, D], f32, tag="yq")
                nc.vector.tensor_copy(yq[:], psy[:])
                nc.sync.dma_start(yt[q], yq[:])

    nc.compile()
    return nc


def _route(expert_idx):
    idx = np.asarray(expert_idx).astype(np.int64)
    order = np.argsort(idx, kind="stable")
    counts = np.bincount(idx, minlength=E)
    starts = np.zeros(E + 1, dtype=np.int64)
    starts[1:] = np.cumsum(counts)
    return order, starts, counts


def _quant_pack(w_cols, scale_cols):
    """w_cols [128, 4096] f32, scale_cols [128, 4096] -> int8."""
    q = np.rint(w_cols / scale_cols)
    np.clip(q, -127, 127, out=q)
    return q.astype(np.int8)


def _pack_inputs(x, fc1_w, fc2_w, order, starts, C):
    import ml_dtypes

    bf16 = ml_dtypes.bfloat16
    blocks = C // CB
    nblk = EPC * blocks
    nquad = nblk // QUAD

    # per-channel scales: s1[e, d] = absmax_h W1[e, h, d] / 127 (folded into
    # x packing), s2[e, d] = absmax_h W2[e, d, h] / 127 (folded into unpack)
    s1 = np.abs(fc1_w).max(axis=1) / 127.0 + 1e-30       # [E, D]
    s2 = np.abs(fc2_w).max(axis=2) / 127.0 + 1e-30       # [E, D]

    in_maps = []
    for core in range(NCORES):
        wh = np.empty((EPC, 128, WQCOLS), np.int8)
        xh = np.zeros((128, nblk * DC * CB), np.float32)
        for s in range(EPC):
            e = core * EPC + s
            # W1T cols: col dc*H + h = W1[h, dc*128+p]; scale s1[e, dc*128+p]
            w1t = np.ascontiguousarray(fc1_w[e].T).reshape(DC, 128, H)
            w1c = w1t.transpose(1, 0, 2).reshape(128, DC * H)
            sc1 = np.repeat(s1[e].reshape(DC, 128).T[:, :, None], H, axis=2)
            # W2T cols: col hc*D + d = W2[d, hc*128+p]; scale s2[e, d]
            w2t = np.ascontiguousarray(fc2_w[e].T).reshape(HC, 128, D)
            w2c = w2t.transpose(1, 0, 2).reshape(128, HC * D)
            sc2 = np.broadcast_to(s2[e][None, None, :], (128, HC, D))
            wh[s, :, :4096] = _quant_pack(w1c, sc1.reshape(128, DC * H))
            wh[s, :, 4096:] = _quant_pack(w2c, sc2.reshape(128, HC * D))

            toks = order[starts[e]:starts[e + 1]]
            n = len(toks)
            if n:
                xs = x[toks] * s1[e][None, :]            # fold W1 scales
                xte = np.ascontiguousarray(xs.T).reshape(DC, 128, n)
                for b in range(blocks):
                    lo, hi = b * CB, min(n, (b + 1) * CB)
                    if lo >= hi:
                        break
                    for c in range(DC):
                        base = ((s * blocks + b) * DC + c) * CB
                        xh[:, base:base + hi - lo] = xte[c][:, lo:hi]
        in_maps.append({
            "wq": wh,
            "xt": xh.astype(bf16),
            "idt": np.eye(128, dtype=np.float32).astype(bf16),
        })
    return in_maps


def _unpack_outputs(results, order, starts, C, out_dtype):
    fc2_scale = _unpack_outputs._s2
    blocks = C // CB
    out = np.zeros((T, D), out_dtype)
    for core in range(NCORES):
        yh = np.asarray(results[core]["yt"], np.float32)   # [nquad, 128, D]
        yh = yh.reshape(-1, CB, D)                         # [nblk, CB, D]
        for s in range(EPC):
            e = core * EPC + s
            toks = order[starts[e]:starts[e + 1]]
            n = len(toks)
            for b in range(blocks):
                lo, hi = b * CB, min(n, (b + 1) * CB)
                if lo >= hi:
                    break
                out[toks[lo:hi]] = (yh[s * blocks + b, :hi - lo]
                                    * fc2_scale[e][None, :])
    return out


def kernel(x, expert_idx, fc1_w, fc2_w):
    from concourse.bass_utils import run_bass_kernel_spmd

    x = np.asarray(x, dtype=np.float32)
    fc1_w = np.asarray(fc1_w, dtype=np.float32)
    fc2_w = np.asarray(fc2_w, dtype=np.float32)

    order, starts, counts = _route(expert_idx)
    C = max(CB, int(-(-int(counts.max()) // CB) * CB))

    if C not in _prog_cache:
        _prog_cache[C] = _build_program(C)
    nc = _prog_cache[C]

    in_maps = _pack_inputs(x, fc1_w, fc2_w, order, starts, C)
    _unpack_outputs._s2 = np.abs(fc2_w).max(axis=2) / 127.0 + 1e-30
    res = run_bass_kernel_spmd(nc, in_maps, list(range(NCORES)))
    return _unpack_outputs(res.results, order, starts, C, np.float32)


# revision 17
# speedup vs baseline: 2.6002x; 1.8818x over previous
"""MoE expert-FFN kernel for Trainium2, expert-parallel across 8 NeuronCores.

Problem: out[t] = silu(x[t] @ W1[e_t]^T) @ W2[e_t]^T with
  E=64 experts, D=512, H=1024, T=256 tokens.

Memory-bound on expert weights. Strategy:
  - Core c owns experts [8c, 8c+8); host routes tokens (all-to-all on host),
    padding each expert's tokens to capacity C (=32 here).
  - Weights are stored in HBM as fp8 e3m4 with per-channel scales that never
    touch the device: W1's per-d scales are folded into the packed x copies,
    W2's per-d scales into host output unpacking. 8.4MB/core vs 33.5MB fp32.
  - Matmuls run mixed-precision (bf16 stationary activations x fp8e3 moving
    weights, fp32 PSUM), verified exact on HW including fp8 subnormals.
  - Four 32-token expert blocks are stacked on PSUM partitions (offsets
    0/32/64/96), engaging 128x32 column tiling so the four blocks' weight
    streams can overlap in the PE array. One ACT Silu per 4-expert quad,
    8 batched [128,128] PE transposes, then fc2 back to [tok, D].
"""

import numpy as np

E, D, H, T = 64, 512, 1024, 256
NCORES = 8
EPC = E // NCORES          # experts per core
DC = D // 128              # 4 d-chunks
HC = H // 128              # 8 h-chunks
CB = 32                    # token block (PSUM partition-stacking granularity)
QUAD = 4                   # token blocks per PSUM quad

W2_INT8 = False            # fallback: W2 as int8 + on-device cast to bf16

WQCOLS = 8192              # per-expert weight columns (W1 4096 | W2 4096)
FP8_MAX = 15.0             # e3m4 absmax target
# engine split points for the optional int8->bf16 W2 cast (DVE / ACT / Pool)
CAST_SPLITS = (4096, 4992, 7424, 8192)

_prog_cache = {}


def _build_program(C):
    import concourse.mybir as mybir
    import concourse.tile as tile
    from concourse import bacc

    f32 = mybir.dt.float32
    bf16 = mybir.dt.bfloat16
    e3 = mybir.dt.float8e3
    i8 = mybir.dt.int8
    Act = mybir.ActivationFunctionType

    blocks = C // CB
    nblk = EPC * blocks            # token blocks per core
    nquad = (nblk + QUAD - 1) // QUAD
    assert nblk % QUAD == 0, "token blocks must tile into quads"

    nc = bacc.Bacc("TRN2", target_bir_lowering=False, debug=False)

    wq = nc.dram_tensor("wq", [EPC, 128, WQCOLS],
                        i8 if W2_INT8 else e3, kind="ExternalInput")
    xt = nc.dram_tensor("xt", [128, nblk * DC * CB], bf16, kind="ExternalInput")
    idt = nc.dram_tensor("idt", [128, 128], bf16, kind="ExternalInput")
    yt = nc.dram_tensor("yt", [nquad, 128, D], f32, kind="ExternalOutput")

    with tile.TileContext(nc) as tc:
        with (
            tc.tile_pool(name="wqpool", bufs=8) as wqpool,
            tc.tile_pool(name="wbpool", bufs=8) as wbpool,
            tc.tile_pool(name="xpool", bufs=1) as xpool,
            tc.tile_pool(name="cpool", bufs=1) as cpool,
            tc.tile_pool(name="hpool", bufs=2) as hpool,
            tc.tile_pool(name="ypool", bufs=2) as ypool,
            tc.tile_pool(name="psh", bufs=2, space="PSUM") as pshp,
            tc.tile_pool(name="pst", bufs=2, space="PSUM") as pstp,
            tc.tile_pool(name="psy", bufs=2, space="PSUM") as psyp,
        ):
            ident = cpool.tile([128, 128], bf16)
            nc.sync.dma_start(ident[:], idt[:])
            xall = xpool.tile([128, nblk * DC * CB], bf16)
            nc.sync.dma_start(xall[:], xt[:])

            w1_of = {}           # expert slot -> fc1 weight tile (fp8e3)
            w2_of = {}           # expert slot -> fc2 weight tile

            def load_expert(s):
                if not W2_INT8:
                    wqs = wqpool.tile([128, WQCOLS], e3, tag="wq")
                    nc.sync.dma_start(wqs[:], wq[s])
                    w1_of[s] = wqs[:, :4096]
                    w2_of[s] = wqs[:, 4096:]
                    return
                wqs = wqpool.tile([128, WQCOLS], i8, tag="wq")
                nc.sync.dma_start(wqs[:], wq[s])
                w1_of[s] = wqs[:, :4096].bitcast(e3)
                wb = wbpool.tile([128, 4096], bf16, tag="wb")
                a, b, c, d = CAST_SPLITS
                nc.vector.tensor_copy(wb[:, :b - a], wqs[:, a:b])
                nc.scalar.activation(wb[:, b - a:c - a], wqs[:, b:c], Act.Copy)
                nc.gpsimd.tensor_copy(wb[:, c - a:], wqs[:, c:d])
                w2_of[s] = wb[:]

            # token block tb = (expert slot s, block b); x block index
            tbs = [(s, b) for s in range(EPC) for b in range(blocks)]

            for q in range(nquad):
                qtbs = tbs[q * QUAD:(q + 1) * QUAD]
                for s, b in qtbs:
                    if s not in w1_of:
                        load_expert(s)

                # fc1: psh[32j+i, h] = sum_d xs[d, tok i of tb j] * W1T[d, h]
                psh = pshp.tile([128, H], f32, tag="psh")
                for dc in range(DC):
                    for nh in range(2):
                        for j, (s, b) in enumerate(qtbs):
                            xoff = ((s * blocks + b) * DC + dc) * CB
                            nc.tensor.matmul(
                                psh[32 * j:32 * (j + 1),
                                    nh * 512:(nh + 1) * 512],
                                xall[:, xoff:xoff + CB],
                                w1_of[s][:, dc * H + nh * 512:
                                         dc * H + (nh + 1) * 512],
                                start=(dc == 0),
                                stop=(dc == DC - 1),
                                tile_position=(0, 32 * j),
                            )

                # silu on the whole quad -> bf16
                hq = hpool.tile([128, H], bf16, tag="hq")
                nc.scalar.activation(hq[:], psh[:], Act.Silu)

                # transpose: hq [tokq, h] -> htq [h, tokq], 8 chunks of 128
                pst = pstp.tile([128, H], bf16, tag="pst")
                for hc in range(HC):
                    nc.tensor.transpose(
                        pst[:, hc * 128:(hc + 1) * 128],
                        hq[:, hc * 128:(hc + 1) * 128],
                        ident[:],
                    )
                htq = hpool.tile([128, H], bf16, tag="htq")
                nc.vector.tensor_copy(htq[:], pst[:])

                # fc2: psy[32j+i, d] = sum_h htq[h, 32j+i] * W2T[h, d]
                psy = psyp.tile([128, D], f32, tag="psy")
                for hc in range(HC):
                    for j, (s, b) in enumerate(qtbs):
                        nc.tensor.matmul(
                            psy[32 * j:32 * (j + 1), :],
                            htq[:, hc * 128 + 32 * j:hc * 128 + 32 * (j + 1)],
                            w2_of[s][:, hc * D:(hc + 1) * D],
                            start=(hc == 0),
                            stop=(hc == HC - 1),
                            tile_position=(0, 32 * j),
                        )

                yq = ypool.tile([128, D], f32, tag="yq")
                nc.vector.tensor_copy(yq[:], psy[:])
                nc.sync.dma_start(yt[q], yq[:])

    nc.compile()
    return nc


def _route(expert_idx):
    idx = np.asarray(expert_idx).astype(np.int64)
    order = np.argsort(idx, kind="stable")
    counts = np.bincount(idx, minlength=E)
    starts = np.zeros(E + 1, dtype=np.int64)
    starts[1:] = np.cumsum(counts)
    return order, starts, counts


def _pack_inputs(x, fc1_w, fc2_w, order, starts, C):
    import ml_dtypes

    bf16 = ml_dtypes.bfloat16
    e3 = ml_dtypes.float8_e3m4
    blocks = C // CB
    nblk = EPC * blocks

    # per-channel scales: s1[e, d] (folded into x packing), s2[e, d]
    # (folded into host output unpacking)
    s1 = np.abs(fc1_w).max(axis=1) / FP8_MAX + 1e-30     # [E, D]
    if W2_INT8:
        s2 = np.abs(fc2_w).max(axis=2) / 127.0 + 1e-30   # [E, D]
    else:
        s2 = np.abs(fc2_w).max(axis=2) / FP8_MAX + 1e-30
    _unpack_outputs._s2 = s2

    in_maps = []
    for core in range(NCORES):
        wh = np.empty((EPC, 128, WQCOLS), e3 if not W2_INT8 else np.int8)
        xh = np.zeros((128, nblk * DC * CB), np.float32)
        for s in range(EPC):
            e = core * EPC + s
            # W1T cols: col dc*H + h = W1[h, dc*128+p]; scale s1[e, dc*128+p]
            w1t = np.ascontiguousarray(fc1_w[e].T).reshape(DC, 128, H)
            w1c = w1t.transpose(1, 0, 2).reshape(128, DC * H)
            sc1 = np.repeat(s1[e].reshape(DC, 128).T[:, :, None], H, axis=2)
            # W2T cols: col hc*D + d = W2[d, hc*128+p]; scale s2[e, d]
            w2t = np.ascontiguousarray(fc2_w[e].T).reshape(HC, 128, D)
            w2c = w2t.transpose(1, 0, 2).reshape(128, HC * D)
            sc2 = np.broadcast_to(s2[e][None, None, :], (128, HC, D))
            w1s = w1c / sc1.reshape(128, DC * H)
            w2s = w2c / sc2.reshape(128, HC * D)
            if not W2_INT8:
                wh[s, :, :4096] = w1s.astype(e3)
                wh[s, :, 4096:] = w2s.astype(e3)
            else:
                wh[s, :, :4096] = w1s.astype(e3).view(np.int8)
                wh[s, :, 4096:] = np.clip(np.rint(w2s), -127, 127
                                          ).astype(np.int8)

            toks = order[starts[e]:starts[e + 1]]
            n = len(toks)
            if n:
                xs = x[toks] * s1[e][None, :]            # fold W1 scales
                xte = np.ascontiguousarray(xs.T).reshape(DC, 128, n)
                for b in range(blocks):
                    lo, hi = b * CB, min(n, (b + 1) * CB)
                    if lo >= hi:
                        break
                    for c in range(DC):
                        base = ((s * blocks + b) * DC + c) * CB
                        xh[:, base:base + hi - lo] = xte[c][:, lo:hi]
        in_maps.append({
            "wq": wh,
            "xt": xh.astype(bf16),
            "idt": np.eye(128, dtype=np.float32).astype(bf16),
        })
    return in_maps


def _unpack_outputs(results, order, starts, C, out_dtype):
    fc2_scale = _unpack_outputs._s2
    blocks = C // CB
    out = np.zeros((T, D), out_dtype)
    for core in range(NCORES):
        yh = np.asarray(results[core]["yt"], np.float32)   # [nquad, 128, D]
        yh = yh.reshape(-1, CB, D)                         # [nblk, CB, D]
        for s in range(EPC):
            e = core * EPC + s
            toks = order[starts[e]:starts[e + 1]]
            n = len(toks)
            for b in range(blocks):
                lo, hi = b * CB, min(n, (b + 1) * CB)
                if lo >= hi:
                    break
                out[toks[lo:hi]] = (yh[s * blocks + b, :hi - lo]
                                    * fc2_scale[e][None, :])
    return out


def kernel(x, expert_idx, fc1_w, fc2_w):
    from concourse.bass_utils import run_bass_kernel_spmd

    x = np.asarray(x, dtype=np.float32)
    fc1_w = np.asarray(fc1_w, dtype=np.float32)
    fc2_w = np.asarray(fc2_w, dtype=np.float32)

    order, starts, counts = _route(expert_idx)
    C = max(CB, int(-(-int(counts.max()) // CB) * CB))

    if C not in _prog_cache:
        _prog_cache[C] = _build_program(C)
    nc = _prog_cache[C]

    in_maps = _pack_inputs(x, fc1_w, fc2_w, order, starts, C)
    res = run_bass_kernel_spmd(nc, in_maps, list(range(NCORES)))
    return _unpack_outputs(res.results, order, starts, C, np.float32)


# revision 24
# speedup vs baseline: 2.6784x; 1.0301x over previous
"""MoE expert-FFN kernel for Trainium2, expert-parallel across 8 NeuronCores.

Problem: out[t] = silu(x[t] @ W1[e_t]^T) @ W2[e_t]^T with
  E=64 experts, D=512, H=1024, T=256 tokens.

Memory-bound on expert weights. Strategy:
  - Core c owns experts [8c, 8c+8); host routes tokens (all-to-all on host),
    padding each expert's tokens to capacity C (=32 here).
  - Weights are stored in HBM as fp8 e3m4 with per-channel scales that never
    touch the device: W1's per-d scales are folded into the packed x copies,
    W2's per-d scales into host output unpacking. 8.4MB/core vs 33.5MB fp32.
  - Matmuls run mixed-precision (bf16 stationary activations x fp8e3 moving
    weights, fp32 PSUM), verified exact on HW including fp8 subnormals.
  - Four 32-token expert blocks are stacked on PSUM partitions (offsets
    0/32/64/96), engaging 128x32 column tiling so the four blocks' weight
    streams can overlap in the PE array. One ACT Silu per 4-expert quad,
    8 batched [128,128] PE transposes, then fc2 back to [tok, D].
"""

import numpy as np

E, D, H, T = 64, 512, 1024, 256
NCORES = 8
EPC = E // NCORES          # experts per core
DC = D // 128              # 4 d-chunks
HC = H // 128              # 8 h-chunks
CB = 32                    # token block (PSUM partition-stacking granularity)
QUAD = 4                   # token blocks per PSUM quad

W2_INT8 = False            # fallback: W2 as int8 + on-device cast to bf16

WQCOLS = 8192              # per-expert weight columns (W1 4096 | W2 4096)
FP8_MAX = 15.0             # e3m4 absmax target
# engine split points for the optional int8->bf16 W2 cast (DVE / ACT / Pool)
CAST_SPLITS = (4096, 4992, 7424, 8192)

_prog_cache = {}


def _build_program(C):
    import concourse.mybir as mybir
    import concourse.tile as tile
    from concourse import bacc

    f32 = mybir.dt.float32
    bf16 = mybir.dt.bfloat16
    e3 = mybir.dt.float8e3
    i8 = mybir.dt.int8
    Act = mybir.ActivationFunctionType

    blocks = C // CB
    nblk = EPC * blocks            # token blocks per core
    nquad = (nblk + QUAD - 1) // QUAD
    assert nblk % QUAD == 0, "token blocks must tile into quads"

    nc = bacc.Bacc("TRN2", target_bir_lowering=False, debug=False)

    # experts packed in side-by-side pairs for 2MB DMAs
    wq = nc.dram_tensor("wq", [EPC // 2, 128, 2 * WQCOLS],
                        i8 if W2_INT8 else e3, kind="ExternalInput")
    xt = nc.dram_tensor("xt", [128, nblk * DC * CB], bf16, kind="ExternalInput")
    idt = nc.dram_tensor("idt", [128, 128], bf16, kind="ExternalInput")
    yt = nc.dram_tensor("yt", [nquad, 128, D], bf16, kind="ExternalOutput")

    with tile.TileContext(nc) as tc:
        with (
            tc.tile_pool(name="wqpool", bufs=8) as wqpool,
            tc.tile_pool(name="wbpool", bufs=8) as wbpool,
            tc.tile_pool(name="xpool", bufs=1) as xpool,
            tc.tile_pool(name="cpool", bufs=1) as cpool,
            tc.tile_pool(name="hpool", bufs=2) as hpool,
            tc.tile_pool(name="ypool", bufs=2) as ypool,
            tc.tile_pool(name="psh", bufs=2, space="PSUM") as pshp,
            tc.tile_pool(name="pst", bufs=2, space="PSUM") as pstp,
            tc.tile_pool(name="psy", bufs=2, space="PSUM") as psyp,
        ):
            # ident + x go first on the ACT ring; weight pair DMAs alternate
            # between the two HWDGE rings (SP / ACT) so transfers overlap
            ident = cpool.tile([128, 128], bf16)
            nc.scalar.dma_start(ident[:], idt[:])
            xall = xpool.tile([128, nblk * DC * CB], bf16)
            nc.scalar.dma_start(xall[:], xt[:])

            w1_of = {}           # expert slot -> fc1 weight tile (fp8e3)
            w2_of = {}           # expert slot -> fc2 weight tile

            def load_pair(p):
                eng = nc.sync if p % 2 == 0 else nc.scalar
                if not W2_INT8:
                    wqs = wqpool.tile([128, 2 * WQCOLS], e3, tag="wq")
                    eng.dma_start(wqs[:], wq[p])
                    for k in range(2):
                        w1_of[2 * p + k] = wqs[:, k * WQCOLS:
                                               k * WQCOLS + 4096]
                        w2_of[2 * p + k] = wqs[:, k * WQCOLS + 4096:
                                               (k + 1) * WQCOLS]
                    return
                wqs = wqpool.tile([128, 2 * WQCOLS], i8, tag="wq")
                eng.dma_start(wqs[:], wq[p])
                a, b, c, d = CAST_SPLITS
                for k in range(2):
                    o = k * WQCOLS
                    w1_of[2 * p + k] = wqs[:, o:o + 4096].bitcast(e3)
                    wb = wbpool.tile([128, 4096], bf16, tag="wb")
                    nc.vector.tensor_copy(wb[:, :b - a], wqs[:, o + a:o + b])
                    nc.scalar.activation(wb[:, b - a:c - a],
                                         wqs[:, o + b:o + c], Act.Copy)
                    nc.gpsimd.tensor_copy(wb[:, c - a:], wqs[:, o + c:o + d])
                    w2_of[2 * p + k] = wb[:]

            # token block tb = (expert slot s, block b); x block index
            tbs = [(s, b) for s in range(EPC) for b in range(blocks)]

            for q in range(nquad):
                qtbs = tbs[q * QUAD:(q + 1) * QUAD]
                for s, b in qtbs:
                    if s not in w1_of:
                        load_pair(s // 2)

                # fc1: psh[32j+i, h] = sum_d xs[d, tok i of tb j] * W1T[d, h]
                psh = pshp.tile([128, H], f32, tag="psh")
                for dc in range(DC):
                    for nh in range(2):
                        for j, (s, b) in enumerate(qtbs):
                            xoff = ((s * blocks + b) * DC + dc) * CB
                            nc.tensor.matmul(
                                psh[32 * j:32 * (j + 1),
                                    nh * 512:(nh + 1) * 512],
                                xall[:, xoff:xoff + CB],
                                w1_of[s][:, dc * H + nh * 512:
                                         dc * H + (nh + 1) * 512],
                                start=(dc == 0),
                                stop=(dc == DC - 1),
                                tile_position=(0, 32 * j),
                            )

                # silu on the whole quad -> bf16
                hq = hpool.tile([128, H], bf16, tag="hq")
                nc.scalar.activation(hq[:], psh[:], Act.Silu)

                # transpose: hq [tokq, h] -> htq [h, tokq], 8 chunks of 128
                pst = pstp.tile([128, H], bf16, tag="pst")
                for hc in range(HC):
                    nc.tensor.transpose(
                        pst[:, hc * 128:(hc + 1) * 128],
                        hq[:, hc * 128:(hc + 1) * 128],
                        ident[:],
                    )
                htq = hpool.tile([128, H], bf16, tag="htq")
                nc.vector.tensor_copy(htq[:], pst[:])

                # fc2: psy[32j+i, d] = sum_h htq[h, 32j+i] * W2T[h, d]
                psy = psyp.tile([128, D], f32, tag="psy")
                for hc in range(HC):
                    for j, (s, b) in enumerate(qtbs):
                        nc.tensor.matmul(
                            psy[32 * j:32 * (j + 1), :],
                            htq[:, hc * 128 + 32 * j:hc * 128 + 32 * (j + 1)],
                            w2_of[s][:, hc * D:(hc + 1) * D],
                            start=(hc == 0),
                            stop=(hc == HC - 1),
                            tile_position=(0, 32 * j),
                        )

                yq = ypool.tile([128, D], bf16, tag="yq")
                nc.vector.tensor_copy(yq[:], psy[:])
                nc.scalar.dma_start(yt[q], yq[:])

    nc.compile()
    return nc


def _route(expert_idx):
    idx = np.asarray(expert_idx).astype(np.int64)
    order = np.argsort(idx, kind="stable")
    counts = np.bincount(idx, minlength=E)
    starts = np.zeros(E + 1, dtype=np.int64)
    starts[1:] = np.cumsum(counts)
    return order, starts, counts


def _pack_inputs(x, fc1_w, fc2_w, order, starts, C):
    import ml_dtypes

    bf16 = ml_dtypes.bfloat16
    e3 = ml_dtypes.float8_e3m4
    blocks = C // CB
    nblk = EPC * blocks

    # per-channel scales: s1[e, d] (folded into x packing), s2[e, d]
    # (folded into host output unpacking)
    s1 = np.abs(fc1_w).max(axis=1) / FP8_MAX + 1e-30     # [E, D]
    if W2_INT8:
        s2 = np.abs(fc2_w).max(axis=2) / 127.0 + 1e-30   # [E, D]
    else:
        s2 = np.abs(fc2_w).max(axis=2) / FP8_MAX + 1e-30
    _unpack_outputs._s2 = s2

    in_maps = []
    for core in range(NCORES):
        wh = np.empty((EPC // 2, 128, 2 * WQCOLS),
                      e3 if not W2_INT8 else np.int8)
        xh = np.zeros((128, nblk * DC * CB), np.float32)
        for s in range(EPC):
            e = core * EPC + s
            # W1T cols: col dc*H + h = W1[h, dc*128+p]; scale s1[e, dc*128+p]
            w1t = np.ascontiguousarray(fc1_w[e].T).reshape(DC, 128, H)
            w1c = w1t.transpose(1, 0, 2).reshape(128, DC * H)
            sc1 = np.repeat(s1[e].reshape(DC, 128).T[:, :, None], H, axis=2)
            # W2T cols: col hc*D + d = W2[d, hc*128+p]; scale s2[e, d]
            w2t = np.ascontiguousarray(fc2_w[e].T).reshape(HC, 128, D)
            w2c = w2t.transpose(1, 0, 2).reshape(128, HC * D)
            sc2 = np.broadcast_to(s2[e][None, None, :], (128, HC, D))
            w1s = w1c / sc1.reshape(128, DC * H)
            w2s = w2c / sc2.reshape(128, HC * D)
            o = (s % 2) * WQCOLS
            if not W2_INT8:
                wh[s // 2, :, o:o + 4096] = w1s.astype(e3)
                wh[s // 2, :, o + 4096:o + WQCOLS] = w2s.astype(e3)
            else:
                wh[s // 2, :, o:o + 4096] = w1s.astype(e3).view(np.int8)
                wh[s // 2, :, o + 4096:o + WQCOLS] = np.clip(
                    np.rint(w2s), -127, 127).astype(np.int8)

            toks = order[starts[e]:starts[e + 1]]
            n = len(toks)
            if n:
                xs = x[toks] * s1[e][None, :]            # fold W1 scales
                xte = np.ascontiguousarray(xs.T).reshape(DC, 128, n)
                for b in range(blocks):
                    lo, hi = b * CB, min(n, (b + 1) * CB)
                    if lo >= hi:
                        break
                    for c in range(DC):
                        base = ((s * blocks + b) * DC + c) * CB
                        xh[:, base:base + hi - lo] = xte[c][:, lo:hi]
        in_maps.append({
            "wq": wh,
            "xt": xh.astype(bf16),
            "idt": np.eye(128, dtype=np.float32).astype(bf16),
        })
    return in_maps


def _unpack_outputs(results, order, starts, C, out_dtype):
    fc2_scale = _unpack_outputs._s2
    blocks = C // CB
    out = np.zeros((T, D), out_dtype)
    for core in range(NCORES):
        yh = np.asarray(results[core]["yt"], np.float32)   # [nquad, 128, D]
        yh = yh.reshape(-1, CB, D)                         # [nblk, CB, D]
        for s in range(EPC):
            e = core * EPC + s
            toks = order[starts[e]:starts[e + 1]]
            n = len(toks)
            for b in range(blocks):
                lo, hi = b * CB, min(n, (b + 1) * CB)
                if lo >= hi:
                    break
                out[toks[lo:hi]] = (yh[s * blocks + b, :hi - lo]
                                    * fc2_scale[e][None, :])
    return out


def kernel(x, expert_idx, fc1_w, fc2_w):
    from concourse.bass_utils import run_bass_kernel_spmd

    x = np.asarray(x, dtype=np.float32)
    fc1_w = np.asarray(fc1_w, dtype=np.float32)
    fc2_w = np.asarray(fc2_w, dtype=np.float32)

    order, starts, counts = _route(expert_idx)
    C = max(CB, int(-(-int(counts.max()) // CB) * CB))

    if C not in _prog_cache:
        _prog_cache[C] = _build_program(C)
    nc = _prog_cache[C]

    in_maps = _pack_inputs(x, fc1_w, fc2_w, order, starts, C)
    res = run_bass_kernel_spmd(nc, in_maps, list(range(NCORES)))
    return _unpack_outputs(res.results, order, starts, C, np.float32)
